# revision 44
# baseline (speedup 1.0000x reference)
"""Bass/Trainium2 kernel for nn_GCNN_61615600828570 (gated GCNN message passing).

Self-contained: hardcodes shapes/sharding. 8 NeuronCores, sharded as
(batch b, arc-direction) pairs; pair AllGather between the two GCN blocks.

Key performance structure:
- adjacency in fp8e4 (0/1 exact), all 10 edge types SBUF-resident,
  loaded once per rep
- arc aggregation uses fp8 DoubleRow matmuls (K=256 packing, 0.5
  cycles/col): p is split into fp8 hi + lo parts (hi+lo ~ bf16 accurate)
- the inter-block exchange is a single pair AllGather of the block-0
  aggregate (both slots summed locally), instead of AllReduce

kernel(**inputs) takes the FULL inputs (numpy, dtypes as in setup_inputs)
and returns the FULL (B, L, E) float32 output.
"""
import numpy as np
import ml_dtypes

import concourse.bass as bass
import concourse.mybir as mybir
import concourse.tile as tile
from concourse import bacc
from concourse.bass_utils import run_bass_kernel_spmd
from concourse.masks import make_identity

F32 = mybir.dt.float32
BF16 = mybir.dt.bfloat16
FP8 = mybir.dt.float8e4
BF = ml_dtypes.bfloat16
E4 = ml_dtypes.float8_e4m3

B, L, E, D = 4, 1024, 140, 140
NE, NU, NB = 10, 4, 2
ND = NU + 1
N1 = D + 1            # 141: D outputs + gate column
LT = L // 128         # 8 l-tiles
KT = L // 128         # 8 contraction tiles for arc
E0 = 128              # first x~ k-slab rows
E1 = E + 1 - E0       # 13: remaining e rows + ones row
NCORES = 8
PAIRS = [[0, 1], [2, 3], [4, 5], [6, 7]]
NW = ND * N1          # 705: distinct projection columns (types>=NU share)
P_CHUNKS = [(0, 512), (512, NW)]  # psum-bank sized N-chunks
L_GROUPS = [(0, 3), (3, 6), (6, 8)]  # l-tile groups per arc psum bank
DR = mybir.MatmulPerfMode.DoubleRow
NO_CC = False

_NC = None


def _build(reps=1):
    et_ = [min(n, NU) for n in range(NE)]
    nc = bacc.Bacc("TRN2", target_bir_lowering=False, debug=False,
                   num_devices=NCORES)

    am_d = nc.dram_tensor("am", [NE, L, L], FP8, kind="ExternalInput")
    x0_d = nc.dram_tensor("x0", [L, E], F32, kind="ExternalInput")
    xt0a_d = nc.dram_tensor("xt0a", [E0, L], BF16, kind="ExternalInput")
    xt0b_d = nc.dram_tensor("xt0b", [16, L], BF16, kind="ExternalInput")
    w0_d = nc.dram_tensor("w0", [NB, E0, NW], BF16, kind="ExternalInput")
    w1_d = nc.dram_tensor("w1", [NB, 16, NW], BF16, kind="ExternalInput")

    outp_d = nc.dram_tensor("outp", [reps, L, E], F32, kind="ExternalOutput")
    x1o_d = nc.dram_tensor("x1o", [reps, L, E], F32, kind="ExternalOutput")

    cc_in = nc.dram_tensor("cc_in", [L, E], BF16)
    cc_out = nc.dram_tensor("cc_out", [2 * L, E], BF16)

    with tile.TileContext(nc) as tc:
        with (
            tc.tile_pool(name="cst", bufs=1) as cst,
            tc.tile_pool(name="amr", bufs=NE) as amr,
            tc.tile_pool(name="pp", bufs=4) as ppool,
            tc.tile_pool(name="wp", bufs=2) as wp,
            tc.tile_pool(name="xp", bufs=2) as xp,
            tc.tile_pool(name="gp", bufs=8) as gpool,
            tc.tile_pool(name="psarc", bufs=4, space="PSUM") as psarc,
            tc.tile_pool(name="psmm", bufs=4, space="PSUM") as psmm,
        ):
            ident = cst.tile([128, 128], BF16)
            make_identity(nc, ident[:])
            identf = cst.tile([128, 128], F32)
            make_identity(nc, identf[:])

            am_view = am_d.ap().rearrange("n (j p) c -> n p j c", p=128)

            def load_am(n, i):
                t = amr.tile([128, KT * L], FP8, tag="amr", name=f"amr_{n}")
                nc.sync.dma_start(
                    t[:].rearrange("p (j c) -> p j c", c=L), am_view[n])
                return t

            # hi = fp8(p) on ACT; lo = fp8(p - hi) mostly ACT, 1/3 on DVE
            # (gpsimd cannot touch PSUM)
            def emit_p(blk, w0, w1, xt_ap, ph, pl, ms, corder,
                       paired=False):
                seq = ([(c, m) for c in range(len(P_CHUNKS)) for m in ms]
                       if corder else
                       [(c, m) for m in ms for c in range(len(P_CHUNKS))])
                for qi, (c, m) in enumerate(seq):
                    c0, c1 = P_CHUNKS[c]
                    if paired:
                        sa = xt_ap[:, m * 256:m * 256 + 128]
                        sb = xt_ap[0:E1, m * 256 + 128:(m + 1) * 256]
                    else:
                        sa = xt_ap[:, m * 128:(m + 1) * 128]
                        sb = xt_ap[0:E1, L + m * 128:L + (m + 1) * 128]
                    mpool = psmm if m % 2 == 0 else psarc
                    mtag = "pmm" if m % 2 == 0 else "arc"
                    pmm = mpool.tile([128, 512], F32, tag=mtag, name="pmm")
                    nc.tensor.matmul(
                        pmm[:, 0:c1 - c0],
                        sa, w0[blk][:, c0:c1], start=True, stop=False)
                    nc.tensor.matmul(
                        pmm[:, 0:c1 - c0],
                        sb, w1[blk][0:E1, c0:c1], start=False, stop=True)
                    hslice = ph[m // 2][:].rearrange(
                        "p (s w) -> p s w", w=NW)[:, m % 2, c0:c1]
                    lslice = pl[m // 2][:].rearrange(
                        "p (s w) -> p s w", w=NW)[:, m % 2, c0:c1]
                    if blk == 0:
                        nc.scalar.copy(hslice, pmm[:, 0:c1 - c0])
                        nc.vector.tensor_tensor(lslice, pmm[:, 0:c1 - c0],
                                                hslice,
                                                op=mybir.AluOpType.subtract)
                    else:
                        # block-1 p overflows fp8e4m3 range: quantize p/16
                        # (gates use sigmoid(16*arc); host rescales outp)
                        nc.scalar.activation(
                            hslice, pmm[:, 0:c1 - c0],
                            mybir.ActivationFunctionType.Copy, scale=0.0625)
                        nc.vector.scalar_tensor_tensor(
                            out=lslice, in0=pmm[:, 0:c1 - c0], scalar=0.0625,
                            in1=hslice, op0=mybir.AluOpType.mult,
                            op1=mybir.AluOpType.subtract)

            for rep in range(reps):
                # ---- p-phase inputs first so compute starts immediately ----
                xt = xp.tile([128, 2 * L], BF16, tag="xt")
                nc.sync.dma_start(xt[:, 0:128], xt0a_d.ap()[:, 0:128])
                nc.sync.dma_start(xt[:, 128:L], xt0a_d.ap()[:, 128:L])
                nc.sync.dma_start(xt[0:16, L:2 * L], xt0b_d.ap())
                w0 = [wp.tile([E0, NW], BF16, tag="w0", name=f"w0_{i}")
                      for i in range(NB)]
                w1 = [wp.tile([16, NW], BF16, tag="w1", name=f"w1_{i}")
                      for i in range(NB)]
                nc.sync.dma_start(w0[0][:, 0:512], w0_d.ap()[0][:, 0:512])
                nc.sync.dma_start(w1[0][:], w1_d.ap()[0])
                nc.sync.dma_start(w0[0][:, 512:NW], w0_d.ap()[0][:, 512:NW])

                # adjacency: all NE types resident in fp8 across both blocks
                am_res = [load_am(n, n) for n in range(NE)]
                amv = [t[:].rearrange("p (j c) -> p j c", c=L) for t in am_res]

                # residual stream + block-1 weights (needed much later)
                xf = xp.tile([128, LT * E], F32, tag="xf")
                nc.sync.dma_start(xf[:].rearrange("p (t d) -> p t d", t=LT),
                                  x0_d.ap().rearrange("(t p) d -> p t d", p=128))
                nc.sync.dma_start(w0[1][:], w0_d.ap()[1])
                nc.sync.dma_start(w1[1][:], w1_d.ap()[1])

                p_next = None
                for blk in range(NB):
                    # ---- p~ = x~ @ [W | Wg] for all NE types, fp8 hi/lo ----
                    if blk == 0:
                        ph = [ppool.tile([128, 2 * NW], FP8, tag="ph",
                                         name=f"ph_0_{i}") for i in range(4)]
                        pl = [ppool.tile([128, 2 * NW], FP8, tag="pl",
                                         name=f"pl_0_{i}") for i in range(4)]
                        emit_p(0, w0, w1, xt, ph, pl, range(KT), corder=True)
                    else:
                        ph, pl = p_next  # built inside the post-AG chain

                    phv = [t[:].rearrange("p (s w) -> p s w", w=NW) for t in ph]
                    plv = [t[:].rearrange("p (s w) -> p s w", w=NW) for t in pl]

                    # ---- arc aggregation: fp8 DoubleRow, K=256 per matmul ----
                    # two accumulators, one per gating path, so the
                    # read-modify-write chains never hop between engines
                    accD = xp.tile([128, LT * D], F32, tag="accD")
                    accP = xp.tile([128, LT * D], F32, tag="accP")
                    nc.gpsimd.memset(accD[:], 0.0)
                    nc.gpsimd.memset(accP[:], 0.0)

                    GROUPS1 = [(0, 3), (3, 6), (6, 7), (7, 8)]

                    def arc_ngroup(n, gi):
                        g0, g1 = (GROUPS1[gi] if blk == 1 else L_GROUPS[gi])
                        gl = g1 - g0
                        apool, atag = ((psarc, "arc") if (n + gi) % 2 == 0
                                       else (psmm, "pmm"))
                        arc = apool.tile([128, 512], F32, tag=atag,
                                         name="arc")
                        for l in range(g0, g1):
                            off = (l - g0) * N1
                            for i in range(4):
                                nc.tensor.matmul(
                                    arc[:, off:off + N1],
                                    amv[n][:, 2 * i:2 * i + 2,
                                           l * 128:(l + 1) * 128],
                                    phv[i][:, :, et_[n] * N1:
                                           (et_[n] + 1) * N1],
                                    start=(i == 0), stop=False,
                                    perf_mode=DR)
                            for i in range(4):
                                nc.tensor.matmul(
                                    arc[:, off:off + N1],
                                    amv[n][:, 2 * i:2 * i + 2,
                                           l * 128:(l + 1) * 128],
                                    plv[i][:, :, et_[n] * N1:
                                           (et_[n] + 1) * N1],
                                    start=False, stop=(i == 3),
                                    perf_mode=DR)
                        g_sb = gpool.tile([128, 4], F32, tag="g")
                        nc.scalar.activation(
                            g_sb[:, 0:gl], arc[:, D:D + (gl - 1) * N1 + 1:N1],
                            mybir.ActivationFunctionType.Sigmoid,
                            scale=16.0 if blk == 1 else 1.0)
                        for l in range(g0, g1):
                            off = (l - g0) * N1
                            if (n + l) % 3 != 0:
                                # gating on DVE (reads PSUM directly)
                                nc.vector.scalar_tensor_tensor(
                                    out=accD[:, l * D:(l + 1) * D],
                                    in0=arc[:, off:off + D],
                                    scalar=g_sb[:, l - g0:l - g0 + 1],
                                    in1=accD[:, l * D:(l + 1) * D],
                                    op0=mybir.AluOpType.mult,
                                    op1=mybir.AluOpType.add)
                            else:
                                # gating via ACT gated-copy + gpsimd add
                                garc = gpool.tile([128, D], BF16,
                                                  tag="garc", bufs=4)
                                nc.scalar.activation(
                                    garc[:], arc[:, off:off + D],
                                    mybir.ActivationFunctionType.Copy,
                                    scale=g_sb[:, l - g0:l - g0 + 1])
                                nc.gpsimd.tensor_tensor(
                                    accP[:, l * D:(l + 1) * D], garc[:],
                                    accP[:, l * D:(l + 1) * D],
                                    op=mybir.AluOpType.add)

                    def merge_group(gi):
                        g0, g1 = (GROUPS1[gi] if blk == 1 else L_GROUPS[gi])
                        gl = g1 - g0
                        sl = slice(g0 * D, g1 * D)
                        meng = nc.vector if blk == 0 else (
                            nc.vector if gi >= 2 else nc.gpsimd)
                        if blk == 1:
                            acc = xp.tile([128, 3 * D], F32, tag="acc",
                                          bufs=3)
                            meng.tensor_tensor(
                                acc[:, 0:gl * D], accD[:, sl], accP[:, sl],
                                op=mybir.AluOpType.add)
                            nc.sync.dma_start(
                                outp_d.ap()[rep, g0 * 128:g1 * 128, :].rearrange(
                                    "(t p) d -> p t d", p=128),
                                acc[:, 0:gl * D].rearrange(
                                    "p (t d) -> p t d", d=D))
                        else:
                            # merge + stage this l-group's finished acc slice
                            # toward the AllGather while later groups compute
                            accb = gpool.tile([128, 3 * D], BF16, tag="accb",
                                              bufs=3, name="accb")
                            meng.tensor_tensor(
                                accb[:, 0:gl * D], accD[:, sl], accP[:, sl],
                                op=mybir.AluOpType.add)
                            nc.gpsimd.dma_start(
                                cc_in.ap()[g0 * 128:g1 * 128, :].rearrange(
                                    "(t p) d -> p t d", p=128),
                                accb[:, 0:gl * D].rearrange(
                                    "p (t d) -> p t d", d=D))

                    if blk == 0:
                        # type-outer: consume each adjacency type as its DMA
                        # lands instead of stalling every l-group on the
                        # last-arriving types; interleave the merges with the
                        # last type so the AllGather staging starts early
                        for n in range(NE - 1):
                            for gi in range(len(L_GROUPS)):
                                arc_ngroup(n, gi)
                        for gi in range(len(L_GROUPS)):
                            arc_ngroup(NE - 1, gi)
                            merge_group(gi)
                    else:
                        # group-outer: stream outputs per l-group; the last
                        # group is split per l-tile for a fast drain
                        for gi in range(len(L_GROUPS) + 1):
                            for n in range(NE):
                                arc_ngroup(n, gi)
                            merge_group(gi)

                    if blk == 0:
                        # ---- pair exchange: single AllGather of acc ----
                        if NO_CC:
                            nc.gpsimd.dma_start(cc_out.ap()[0:L, :],
                                                cc_in.ap())
                            nc.gpsimd.dma_start(cc_out.ap()[L:2 * L, :],
                                                cc_in.ap())
                        else:
                            nc.gpsimd.collective_compute(
                                "AllGather", mybir.AluOpType.bypass,
                                replica_groups=PAIRS,
                                ins=[cc_in.ap()], outs=[cc_out.ap()])

                        # ---- post-AG chain, pipelined per l-group ----
                        x1 = xp.tile([128, LT * E], F32, tag="xf")
                        xt_n = xp.tile([128, 2 * L], BF16, tag="xt")
                        nc.gpsimd.memset(xt_n[0:32, L:2 * L], 1.0)
                        red0 = xp.tile([128, LT * E], BF16, tag="red0")
                        red1 = xp.tile([128, LT * E], BF16, tag="red1")
                        ph_n = [ppool.tile([128, 2 * NW], FP8, tag="ph",
                                           name=f"ph_1_{i}") for i in range(4)]
                        pl_n = [ppool.tile([128, 2 * NW], FP8, tag="pl",
                                           name=f"pl_1_{i}") for i in range(4)]
                        p_next = (ph_n, pl_n)
                        # reconstruct x1 and its transpose for all l-tiles
                        # first (the p-matmul quantization would otherwise
                        # clog DVE/ACT ahead of later groups' chains), then
                        # emit all block-1 p-matmuls
                        for (g0, g1) in [(0, 1), (1, 2), (2, 3), (3, 6),
                                         (6, 8)]:
                            sl = slice(g0 * E, g1 * E)
                            nc.sync.dma_start(
                                red0[:, sl].rearrange("p (t d) -> p t d", d=E),
                                cc_out.ap()[g0 * 128:g1 * 128, :].rearrange(
                                    "(t p) d -> p t d", p=128))
                            nc.sync.dma_start(
                                red1[:, sl].rearrange("p (t d) -> p t d", d=E),
                                cc_out.ap()[L + g0 * 128:L + g1 * 128, :].rearrange(
                                    "(t p) d -> p t d", p=128))
                            nc.vector.tensor_tensor(
                                x1[:, sl], red0[:, sl], red1[:, sl],
                                op=mybir.AluOpType.add)
                            nc.vector.scalar_tensor_tensor(
                                out=x1[:, sl], in0=x1[:, sl], scalar=0.0,
                                in1=xf[:, sl], op0=mybir.AluOpType.max,
                                op1=mybir.AluOpType.add)
                            for lt in range(g0, g1):
                                # transpose x1 directly (f32 in PSUM), cast
                                # to bf16 on the way out
                                tp = psmm.tile([128, 512], F32, tag="pmm")
                                nc.tensor.transpose(
                                    tp[:, 0:128],
                                    x1[:, lt * E:lt * E + 128], identf[:])
                                nc.tensor.transpose(
                                    tp[0:E - E0, 128:256],
                                    x1[:, lt * E + E0:lt * E + E], identf[:])
                                nc.scalar.copy(
                                    xt_n[:, lt * 128:(lt + 1) * 128],
                                    tp[:, 0:128])
                                nc.scalar.copy(
                                    xt_n[0:E - E0, L + lt * 128:L + (lt + 1) * 128],
                                    tp[0:E - E0, 128:256])
                        nc.gpsimd.dma_start(
                            x1o_d.ap()[rep].rearrange(
                                "(t p) d -> p t d", p=128),
                            x1[:].rearrange("p (t d) -> p t d", d=E))
                        emit_p(1, w0, w1, xt_n, ph_n, pl_n, range(KT),
                               corder=True)
                        xt = xt_n
                        xf = x1

    nc.compile()
    return nc


def _get_nc():
    global _NC
    if _NC is None:
        _NC = _build()
    return _NC


def _prep_inputs(seq_repr, adj, W_in, b_in, W_out, b_out,
                 Wg_in, bg_in, Wg_out, bg_out):
    """Build the 8 per-core input maps (host-side sharding + layout prep)."""
    et = np.minimum(np.arange(NE), NU)
    seq_repr = np.asarray(seq_repr, np.float32)
    adj = np.asarray(adj)

    # x~0^T slabs, shared by all cores of the same b
    xt_by_b = []
    for b in range(B):
        xt = np.concatenate(
            [seq_repr[b], np.ones((L, 1), np.float32)], axis=1).T  # (141, L)
        xt = xt.astype(BF)
        xt0b = np.zeros((16, L), BF)
        xt0b[0:E1] = xt[E0:E + 1]
        xt_by_b.append((np.ascontiguousarray(xt[0:E0]), xt0b))

    # weight slabs per direction: rows = e (140) + bias row; cols = NE*(D+1)
    def wslabs(Wd, bd, Wgd, bgd):
        w = np.zeros((NB, E + 1, NW), np.float32)
        for blk in range(NB):
            for n in range(ND):
                w[blk, 0:E, n * N1:n * N1 + D] = Wd[blk, n]
                w[blk, E, n * N1:n * N1 + D] = bd[blk, n]
                w[blk, 0:E, n * N1 + D] = Wgd[blk, n, :, 0]
                w[blk, E, n * N1 + D] = bgd[blk, n, 0]
        w = w.astype(BF)
        w1 = np.zeros((NB, 16, NW), BF)
        w1[:, 0:E1] = w[:, E0:E + 1]
        return np.ascontiguousarray(w[:, 0:E0]), w1

    w_in0, w_in1 = wslabs(np.asarray(W_in, np.float32), np.asarray(b_in, np.float32),
                          np.asarray(Wg_in, np.float32), np.asarray(bg_in, np.float32))
    w_out0, w_out1 = wslabs(np.asarray(W_out, np.float32), np.asarray(b_out, np.float32),
                            np.asarray(Wg_out, np.float32), np.asarray(bg_out, np.float32))

    in_maps = []
    for c in range(NCORES):
        b, dirn = c // 2, c % 2
        a = adj[b].astype(E4)  # (NE, L, L), 0/1 exact in fp8e4
        if dirn == 0:
            # in-arcs: lhsT tile [m, l] must hold A[l, m] -> transpose
            am = np.ascontiguousarray(a.transpose(0, 2, 1))
            w0, w1 = w_in0, w_in1
        else:
            am = np.ascontiguousarray(a)
            w0, w1 = w_out0, w_out1
        xt0a, xt0b = xt_by_b[b]
        in_maps.append({
            "am": am, "x0": np.ascontiguousarray(seq_repr[b]),
            "xt0a": xt0a, "xt0b": xt0b, "w0": w0, "w1": w1,
        })
    return in_maps


def _combine(results):
    """Host epilogue: x2 = relu(p_in + p_out) + x1 per batch."""
    out = np.empty((B, L, E), np.float32)
    for b in range(B):
        pin = results[2 * b]["outp"][0]
        pout = results[2 * b + 1]["outp"][0]
        x1 = results[2 * b]["x1o"][0]
        out[b] = np.maximum((pin + pout) * 16.0, 0.0) + x1
    return out


def run_on_hw(in_maps, trace=False, **kw):
    nc = _get_nc()
    res = run_bass_kernel_spmd(nc, in_maps, core_ids=list(range(NCORES)),
                               trace=trace, **kw)
    return res


def kernel(**inputs):
    in_maps = _prep_inputs(**inputs)
    # the axon tunnel intermittently drops a worker ("notify failed ...
    # hung up") independent of kernel content; a fresh execution succeeds
    last = None
    for _ in range(3):
        try:
            res = run_on_hw(in_maps)
            return _combine(res.results)
        except Exception as e:
            last = e
    raise last
